# revision 1
# baseline (speedup 1.0000x reference)
"""Llama GQA attention (B=2, S=2048, D=2048, H=16, KVH=4, HD=128), 8-way sharded.

Sharding (tensor-parallel per the hint, plus data-parallel on batch):
  core c in 0..7 -> (b = c // 4, g = c % 4)
  Each core owns one batch element and one KV-head group (1 KV head and its
  4 query heads): it gets Wq/bq cols [512g:512g+512], Wk/Wv/bk/bv cols
  [128g:128g+128], and Wo rows [512g:512g+512].  It computes the full
  attention for its 4 heads and a partial Wo product; the host sums the 4
  partial Wo outputs per batch and concatenates attn_weights shards.
"""
import sys
import numpy as np

B, S, D = 2, 2048, 2048
H, KVH, HD = 16, 4, 128
N_REP = H // KVH
SCALING = HD ** -0.5
GQ = H // KVH  # query heads per group = 4


def _shard_fn_builder(jnp):
    def rotate_half(x):
        x1, x2 = jnp.split(x, 2, axis=-1)
        return jnp.concatenate([-x2, x1], axis=-1)

    def shard_fn(hid, cos, sin, mask, Wq, bq, Wk, bk, Wv, bv, Wo):
        # hid [S, D]; Wq [D, 512]; Wk/Wv [D, 128]; Wo [512, D]; mask [S, S]
        q = (hid @ Wq + bq).reshape(S, GQ, HD).transpose(1, 0, 2)   # [4, S, HD]
        k = (hid @ Wk + bk).reshape(S, HD)                           # [S, HD]
        v = (hid @ Wv + bv).reshape(S, HD)                           # [S, HD]
        q = q * cos[None] + rotate_half(q) * sin[None]
        k = k * cos + rotate_half(k) * sin
        scores = jnp.einsum('hqd,kd->hqk', q, k) * SCALING + mask[None]
        w = jax_softmax(scores)                                      # [4, S, S]
        ctx = jnp.einsum('hqk,kd->hqd', w, v)                        # [4, S, HD]
        ctx = ctx.transpose(1, 0, 2).reshape(S, GQ * HD)             # [S, 512]
        out_partial = ctx @ Wo                                       # [S, D]
        return w, out_partial

    def jax_softmax(x):
        m = jnp.max(x, axis=-1, keepdims=True)
        e = jnp.exp(x - m)
        return e / jnp.sum(e, axis=-1, keepdims=True)

    return shard_fn


def _make_shards(hidden_states, cos, sin, attention_mask,
                 Wq, bq, Wk, bk, Wv, bv, Wo):
    """Stack per-core input shards along a leading axis of size 8."""
    f32 = np.float32
    hids, coss, sins, masks = [], [], [], []
    Wqs, bqs, Wks, bks, Wvs, bvs, Wos = [], [], [], [], [], [], []
    for c in range(8):
        b, g = c // 4, c % 4
        hids.append(hidden_states[b])
        coss.append(cos[b])
        sins.append(sin[b])
        masks.append(attention_mask[b, 0])
        Wqs.append(Wq[:, 512 * g:512 * (g + 1)])
        bqs.append(bq[512 * g:512 * (g + 1)])
        Wks.append(Wk[:, 128 * g:128 * (g + 1)])
        bks.append(bk[128 * g:128 * (g + 1)])
        Wvs.append(Wv[:, 128 * g:128 * (g + 1)])
        bvs.append(bv[128 * g:128 * (g + 1)])
        Wos.append(Wo[512 * g:512 * (g + 1), :])
    stack = lambda xs: np.ascontiguousarray(np.stack(xs)).astype(f32, copy=False)
    return tuple(map(stack, (hids, coss, sins, masks, Wqs, bqs, Wks, bks,
                             Wvs, bvs, Wos)))


def _assemble(w_shards, out_partials):
    """w_shards [8, 4, S, S]; out_partials [8, S, D] -> full outputs."""
    attn_weights = np.empty((B, H, S, S), dtype=np.float32)
    attn_out = np.zeros((B, S, D), dtype=np.float32)
    for c in range(8):
        b, g = c // 4, c % 4
        attn_weights[b, 4 * g:4 * (g + 1)] = w_shards[c]
        attn_out[b] += out_partials[c]
    return attn_out, attn_weights


def _run_device(shards):
    import jax
    import jax.numpy as jnp
    devs = [d for d in jax.devices() if d.platform != 'cpu'][:8]
    if len(devs) < 8:
        raise RuntimeError(f"need 8 accelerator devices, have {len(devs)}")
    shard_fn = _shard_fn_builder(jnp)
    fn = jax.pmap(shard_fn, devices=devs)
    w_shards, out_partials = fn(*shards)
    return np.asarray(w_shards), np.asarray(out_partials)


def _run_cpu(shards):
    """Per-shard computation with numpy (float32 BLAS) — correctness fallback."""
    ws, outs = [], []
    for c in range(8):
        hid, cos, sin, mask, Wq, bq, Wk, bk, Wv, bv, Wo = (a[c] for a in shards)
        q = (hid @ Wq + bq).reshape(S, GQ, HD).transpose(1, 0, 2)
        k = hid @ Wk + bk
        v = hid @ Wv + bv

        def rot(x):
            x1, x2 = x[..., :HD // 2], x[..., HD // 2:]
            return np.concatenate([-x2, x1], axis=-1)

        q = q * cos[None] + rot(q) * sin[None]
        k = k * cos + rot(k) * sin
        scores = np.einsum('hqd,kd->hqk', q, k, optimize=True) * SCALING
        scores += mask[None]
        m = scores.max(axis=-1, keepdims=True)
        e = np.exp(scores - m)
        w = e / e.sum(axis=-1, keepdims=True)
        ctx = np.einsum('hqk,kd->hqd', w, v, optimize=True)
        ctx = ctx.transpose(1, 0, 2).reshape(S, GQ * HD)
        ws.append(w.astype(np.float32))
        outs.append((ctx @ Wo).astype(np.float32))
    return np.stack(ws), np.stack(outs)


def kernel(hidden_states, cos, sin, attention_mask, Wq, bq, Wk, bk, Wv, bv, Wo):
    shards = _make_shards(hidden_states, cos, sin, attention_mask,
                          Wq, bq, Wk, bk, Wv, bv, Wo)
    try:
        w_shards, out_partials = _run_device(shards)
    except Exception as e:  # pragma: no cover — device unavailable/compile issue
        print(f"kernel: device path failed ({type(e).__name__}: {e}); "
              f"falling back to host execution", file=sys.stderr)
        w_shards, out_partials = _run_cpu(shards)
    return _assemble(w_shards, out_partials)
